# revision 1
# baseline (speedup 1.0000x reference)
"""CrossNetMoE forward on 8 Trainium2 NeuronCores (Bass/Tile).

Math (per layer i, E=4 experts, rank R=64, D=1024):
    v = tanh(V_e @ xl)            [B,E,R]
    c = tanh(C_e @ v_e)           [B,E,R]
    g = softmax(Wg_e . xl)        [B,E]
    u = sum_e (g_e * c_e) @ U_e.T + b      (softmax weights sum to 1)
    xl' = tanh(u * x0 + xl)   (last layer: no tanh)

Strategy: pure data-parallel over batch (2048 rows/core), everything kept in
transposed layout [D, B] on-chip so each layer's matmuls consume the previous
layer's output directly as the PE moving operand; softmax over the 4 experts is
done with tiny auxiliary matmuls (partition reduction / broadcast); the final
`u * x0 + xl` keeps u in PSUM: DVE multiplies x0 in place, then an
identity-weight matmul accumulates xl on top (has_written bits stay set), and
ACT evacuates with the tanh. f32r (TF32-like) matmul throughput is 1 col/cycle.

Host side transposes x once and un-transposes the result; weights are packed
host-side into SBUF-image blobs replicated to all cores.
"""
import json
import os
import sys

sys.path.insert(0, "/opt/trn_rl_repo")

import numpy as np

L, E, D, R = 3, 4, 1024, 64
B = 16384
NCORES = 8
BC = B // NCORES          # 2048 rows per core
N = 256                   # batch columns per group (matmul moving free dim)
G = BC // N               # 8 groups per core
NCH = D // 128            # 8 d-chunks

# per-layer weight blob column offsets (f32 columns, [128, COLS_L])
V_OFF = 0                 # 2 groups x 8 chunks x 128
U_OFF = 2048              # 2 kchunks x 8 mchunks x 128
C_OFF = 4096              # 2 groups x 128
W_OFF = 4352              # 8 chunks x 4
B_OFF = 4384              # 8 chunks x 1
COLS_L = 4392
# common blob
I_OFF = 0                 # identity 128
P_OFF = 128               # 2 groups x 128 (partitions 0..3)
O4_OFF = 384              # ones [4,1]
O14_OFF = 385             # ones [1,4]
O44_OFF = 389             # all-ones [4,4] for Z4
COLS_C = 396

_EXEMPT = {"Call"}


def _legalize_json_bytes(raw: bytes) -> bytes:
    """Split multi-wait instructions: walrus allows 1 sync-wait per inst."""
    m = json.loads(raw)
    counter = [0]

    def fix_block(block):
        insts = block.get("instructions")
        if insts is not None:
            out = []
            for inst in insts:
                si = inst.get("sync_info")
                if (
                    si
                    and inst.get("opcode") not in _EXEMPT
                    and len(si.get("on_wait") or []) > 1
                ):
                    for w in si["on_wait"][:-1]:
                        counter[0] += 1
                        out.append(
                            {
                                "name": f"I-waitsplit-{counter[0]}",
                                "opcode": "NoOp",
                                "engine": inst["engine"],
                                "ins": [],
                                "outs": [],
                                "debug": 0,
                                "sync_info": {"on_wait": [w], "on_update": []},
                            }
                        )
                    si["on_wait"] = [si["on_wait"][-1]]
                out.append(inst)
            block["instructions"] = out
        for sub in block.get("blocks") or []:
            fix_block(sub)

    for f in m["functions"]:
        for b in f["blocks"]:
            fix_block(b)
    return json.dumps(m).encode()


def pack_weights(U, V, C, Wg, b):
    """Pack parameters into SBUF-image blobs [128, COLS] (host side)."""
    U, V, C, Wg, b = (np.asarray(a, np.float32) for a in (U, V, C, Wg, b))
    blobs = {}
    p = np.arange(128)
    for l in range(L):
        blob = np.zeros((128, COLS_L), np.float32)
        # Vw: lhsT chunk for vT matmuls: [p=d%128, (g,c,m)]; m -> (e=2g+m//64, r=m%64)
        for g in range(2):
            for c in range(NCH):
                m = np.arange(128)
                # V[l, 2g + m//64, m%64, c*128+p]
                blob[:, V_OFF + (g * 8 + c) * 128 : V_OFF + (g * 8 + c + 1) * 128] = (
                    V[l, 2 * g + m[None, :] // 64, m[None, :] % 64, c * 128 + p[:, None]]
                )
        # Uw: [p=(e,r2)%128 of kchunk, (kc,mc,q)] = U[l, (kc*128+p)//64, mc*128+q, (kc*128+p)%64]
        for kc in range(2):
            for mc in range(NCH):
                q = np.arange(128)
                blob[:, U_OFF + (kc * 8 + mc) * 128 : U_OFF + (kc * 8 + mc + 1) * 128] = (
                    U[l, (kc * 128 + p[:, None]) // 64, mc * 128 + q[None, :], (kc * 128 + p[:, None]) % 64]
                )
        # Cw: blockdiag pairs: [p=(el',r1), (g,j=(el,r2))]
        for g in range(2):
            j = np.arange(128)
            el_p = p[:, None] // 64
            el_j = j[None, :] // 64
            val = C[l, 2 * g + el_j, j[None, :] % 64, p[:, None] % 64]
            blob[:, C_OFF + g * 128 : C_OFF + (g + 1) * 128] = np.where(el_p == el_j, val, 0.0)
        # Ww: [p=d%128, (c,e)]
        for c in range(NCH):
            blob[:, W_OFF + c * 4 : W_OFF + (c + 1) * 4] = Wg[l, :, c * 128 + p]
        # bias
        for c in range(NCH):
            blob[:, B_OFF + c] = b[l, c * 128 + p]
        blobs[f"wl{l}"] = blob
    wc = np.zeros((128, COLS_C), np.float32)
    wc[:, I_OFF : I_OFF + 128] = np.eye(128, dtype=np.float32)
    for g in range(2):
        m = np.arange(128)
        wc[0:4, P_OFF + g * 128 : P_OFF + (g + 1) * 128] = (
            np.arange(4)[:, None] == (2 * g + m[None, :] // 64)
        ).astype(np.float32)
    wc[0:4, O4_OFF] = 1.0
    wc[0:1, O14_OFF : O14_OFF + 4] = 1.0
    wc[0:4, O44_OFF : O44_OFF + 4] = 1.0
    blobs["wc"] = wc
    return blobs


def build_nc(bias_nonzero=False, mode="bf16"):
    import concourse.bass as bass
    import concourse.tile as tile
    from concourse import mybir
    from concourse.tile import add_dep_helper

    f32 = mybir.dt.float32
    f32r = mybir.dt.float32r
    AF = mybir.ActivationFunctionType
    ALU = mybir.AluOpType
    bf16 = mybir.dt.bfloat16
    mt = bf16 if mode == "bf16" else f32r  # dtype of the heavy matmul path

    nc = bass.Bass()
    xT = nc.dram_tensor("xT", [D, BC], mt, kind="ExternalInput")
    xF = nc.dram_tensor("xF", [D, BC], f32, kind="ExternalInput")
    wl = [nc.dram_tensor(f"wl{l}", [128, COLS_L], mt, kind="ExternalInput") for l in range(L)]
    wbd = nc.dram_tensor("wb", [128, L * NCH], f32, kind="ExternalInput") if bias_nonzero else None
    wid = nc.dram_tensor("wid", [128, 388], mt, kind="ExternalInput")
    outT = nc.dram_tensor("outT", [D, BC], f32, kind="ExternalOutput")

    # chain matmuls that share a psum tile so scheduler keeps program order
    last_mm = {}

    def mm(key, out, lhsT, rhs, start, stop):
        inst = nc.tensor.matmul(out, lhsT, rhs, start=start, stop=stop, skip_group_check=True)
        if key in last_mm:
            add_dep_helper(inst.ins, last_mm[key].ins, sync=False, reason="psum order")
        last_mm[key] = inst
        return inst

    with tile.TileContext(nc) as tc:
        with (
            tc.tile_pool(name="wpool", bufs=1) as wpool,
            tc.tile_pool(name="xpool", bufs=1) as xpool,
            tc.tile_pool(name="mid", bufs=1) as mid,
            tc.tile_pool(name="pspool", bufs=1, space="PSUM") as pspool,
            nc.allow_low_precision(reason="f32r/bf16 matmul pipeline (intentional)"),
        ):
            wt = []
            for l in range(L):
                w_l = wpool.tile([128, COLS_L], mt, name=f"wt{l}")
                wt.append(w_l)
            startup_w = [
                (wt[0][:, 0:2048], wl[0][:, 0:2048]),
                (wt[0][:, 4352:COLS_L], wl[0][:, 4352:COLS_L]),
                (wt[0][:, 2048:4352], wl[0][:, 2048:4352]),
            ]
            wb_f32 = None
            if bias_nonzero:
                wbt = wpool.tile([128, L * NCH], f32)
                nc.sync.dma_start(wbt[:], wbd[:])
                wb_f32 = [wbt[:, l * NCH : (l + 1) * NCH] for l in range(L)]
            identm = wpool.tile([128, 388], mt, name="identm")
            ident = identm[:, 0:128]
            Pb = identm[0:4, 128:384]
            ones44 = identm[0:4, 384:388]


            xT_v = xT.rearrange("(c p) b -> p c b", p=128)
            xF_v = xF.rearrange("(c p) b -> p c b", p=128)
            outT_v = outT.rearrange("(c p) b -> p c b", p=128)

            ROT = 4  # groups processed in rotation
            for sup in range(G // ROT):
                gs = [ROT * sup + i for i in range(ROT)]
                st = {}
                for g in gs:
                    x0t = xpool.tile([128, NCH * N], mt, tag="x0", bufs=5, name=f"x0_{g}")
                    nc.sync.dma_start(x0t[:], xT_v[:, :, g * N : (g + 1) * N])
                    if startup_w is not None and g == gs[0]:
                        nc.sync.dma_start(*startup_w[0])
                    x0f = xpool.tile([128, NCH * N], f32, tag="x0f", bufs=5, name=f"x0f_{g}")
                    nc.scalar.dma_start(x0f[:], xF_v[:, :, g * N : (g + 1) * N])
                    st[g] = dict(x0t=x0t, x0f=x0f, xin=x0t)
                if startup_w is not None:
                    for dst, src_ap in startup_w[1:]:
                        nc.sync.dma_start(dst, src_ap)
                    nc.scalar.dma_start(identm[:], wid[:])
                    for l_ in range(1, L):
                        nc.scalar.dma_start(wt[l_][:], wl[l_][:])
                    startup_w = None

                for l in range(L):
                    # ---------- P1 + per-group softmax chain ----------
                    for gi, g in enumerate(gs):
                        S = st[g]
                        xin = S["xin"]
                        pair = gi // 2
                        soff = (gi % 2) * N
                        if gi % 2 == 0:
                            st[("sml", pair)] = pspool.tile(
                                [4, 512], f32, tag="sml", bufs=1, name=f"sml{sup}_{l}_{pair}"
                            )
                        s_ps = st[("sml", pair)]
                        ks = f"s{sup}_{l}_{pair}"
                        kv = f"v{g}_{l}"
                        v_ps = pspool.tile([128, 512], f32, tag="v", bufs=2, name=f"vps{g}_{l}")
                        for c in range(NCH):
                            rhs = xin[:, c * N : (c + 1) * N]
                            mm(kv, v_ps[:, 0:N], wt[l][:, V_OFF + c * 128 : V_OFF + (c + 1) * 128], rhs, start=(c == 0), stop=False)
                            mm(kv, v_ps[:, N : 2 * N], wt[l][:, V_OFF + (8 + c) * 128 : V_OFF + (9 + c) * 128], rhs, start=False, stop=(c == NCH - 1))
                        for c in range(NCH):
                            rhs = xin[:, c * N : (c + 1) * N]
                            mm(ks, s_ps[0:4, soff : soff + N], wt[l][:, W_OFF + c * 4 : W_OFF + (c + 1) * 4], rhs, start=(c == 0 and gi % 2 == 0), stop=(c == NCH - 1))
                        # chain for this group: tanh(v), exp(s), Z4, 1/Z, gates
                        vt = mid.tile([128, 512], mt, tag="vt", bufs=5, name=f"vt{g}_{l}")
                        nc.scalar.activation(vt[:], v_ps[:], AF.Tanh)
                        e_s = mid.tile([4, N], mt, tag="es", bufs=5, name=f"es{g}_{l}")
                        nc.scalar.activation(e_s[:], s_ps[0:4, soff : soff + N], AF.Exp)
                        # Z4: all-ones 4x4 -> each row = sum over experts (overlays s region)
                        mm(ks, s_ps[0:4, soff : soff + N], ones44, e_s[:], start=True, stop=True)
                        rr4 = mid.tile([4, N], mt, tag="rr", bufs=5, name=f"rr{g}_{l}")
                        nc.vector.reciprocal(rr4[:], s_ps[0:4, soff : soff + N])
                        g4 = mid.tile([4, N], mt, tag="g4", bufs=5, name=f"g4{g}_{l}")
                        nc.vector.tensor_mul(g4[:], e_s[:], rr4[:])
                        S["vt"], S["g4"] = vt, g4
                    # ---------- C / gate-broadcast / cg ----------
                    for g in gs:
                        S = st[g]
                        kc = f"c{g}_{l}"
                        c_ps = pspool.tile([128, 512], f32, tag="cb", bufs=1, name=f"cps{g}_{l}")
                        mm(kc, c_ps[:, 0:N], wt[l][:, C_OFF : C_OFF + 128], S["vt"][:, 0:N], start=True, stop=True)
                        mm(kc, c_ps[:, N : 2 * N], wt[l][:, C_OFF + 128 : C_OFF + 256], S["vt"][:, N : 2 * N], start=False, stop=True)
                        ct = mid.tile([128, 512], mt, tag="ct", bufs=4, name=f"ct{g}_{l}")
                        nc.scalar.activation(ct[:], c_ps[:], AF.Tanh)
                        kb = f"b{g}_{l}"
                        b_ps = pspool.tile([128, 512], f32, tag="cb", bufs=1, name=f"bps{g}_{l}")
                        mm(kb, b_ps[:, 0:N], Pb[:, 0:128], S["g4"][:], start=True, stop=True)
                        mm(kb, b_ps[:, N : 2 * N], Pb[:, 128:256], S["g4"][:], start=False, stop=True)
                        cg = mid.tile([128, 512], mt, tag="cg", bufs=4, name=f"cg{g}_{l}")
                        nc.vector.tensor_mul(cg[:], ct[:] if mode == "bf16" else ct[:].bitcast(f32), b_ps[:])
                        S["cg"] = cg
                        if l < L - 1:
                            S["xout"] = xpool.tile([128, NCH * N], mt, tag="xl", bufs=8, name=f"xl{g}_{l}")
                        else:
                            S["xout"] = xpool.tile([128, NCH * N], f32, tag="osb", bufs=2, name=f"osb{g}")
                    # ---------- U matmuls + epilogue (quarter granularity) ----------
                    for g in gs:
                        S = st[g]
                        xin, x0f, cg, xout = S["xin"], S["x0f"], S["cg"], S["xout"]
                        for q in range(4):
                            ku = f"u{g}_{l}_{q}"
                            u_ps = pspool.tile([128, 512], f32, tag="u", bufs=4, name=f"ups{g}_{l}_{q}")
                            for mi, mc in enumerate((2 * q, 2 * q + 1)):
                                col = mi * N
                                for kch in range(2):
                                    mm(
                                        ku,
                                        u_ps[:, col : col + N],
                                        wt[l][:, U_OFF + (kch * 8 + mc) * 128 : U_OFF + (kch * 8 + mc + 1) * 128],
                                        cg[:, kch * N : (kch + 1) * N],
                                        start=(mi == 0 and kch == 0),
                                        stop=(kch == 1),
                                    )
                            if bias_nonzero:
                                for mi, mc in enumerate((2 * q, 2 * q + 1)):
                                    col = mi * N
                                    nc.vector.scalar_tensor_tensor(
                                        u_ps[:, col : col + N],
                                        u_ps[:, col : col + N],
                                        wb_f32[l][:, mc : mc + 1],
                                        x0f[:, mc * N : (mc + 1) * N],
                                        ALU.add,
                                        ALU.mult,
                                    )
                            else:
                                nc.vector.tensor_mul(u_ps[:], u_ps[:], x0f[:, q * 512 : (q + 1) * 512])
                            mm(ku, u_ps[:], ident, xin[:, 2 * q * N : (2 * q + 2) * N], start=False, stop=True)
                            if l < L - 1:
                                nc.scalar.activation(xout[:, q * 512 : (q + 1) * 512], u_ps[:], AF.Tanh)
                            else:
                                nc.scalar.activation(xout[:, q * 512 : (q + 1) * 512], u_ps[:], AF.Copy)
                    for g in gs:
                        st[g]["xin"] = st[g]["xout"]
                for g in gs:
                    ov = outT.rearrange("(h c p) b -> p h c b", p=128, h=2)
                    for hh in range(2):
                        nc.sync.dma_start(
                            ov[:, hh, :, g * N : (g + 1) * N],
                            st[g]["xin"][:, hh * 1024 : (hh + 1) * 1024].rearrange("p (c n) -> p c n", c=4),
                        )

    # walrus wait-budget legalization on serialization
    orig = nc.to_json_bytes
    nc.to_json_bytes = lambda: _legalize_json_bytes(orig())
    return nc


_CACHE = {}


MODE = "bf16"


def kernel(x, U, V, C, Wg, b):
    import ml_dtypes

    x = np.ascontiguousarray(np.asarray(x, np.float32))
    bias_nonzero = bool(np.any(np.asarray(b) != 0))
    key = ("nc", bias_nonzero, MODE)
    if key not in _CACHE:
        _CACHE[key] = build_nc(bias_nonzero, MODE)
    nc = _CACHE[key]
    mnp = ml_dtypes.bfloat16 if MODE == "bf16" else np.float32
    blobs = pack_weights(U, V, C, Wg, b)
    xTfull = np.ascontiguousarray(x.T)  # [D, B]
    wls = {f"wl{l}": np.ascontiguousarray(blobs[f"wl{l}"].astype(mnp)) for l in range(L)}
    wid = np.zeros((128, 388), np.float32)
    wid[:, 0:128] = np.eye(128, dtype=np.float32)
    mcol = np.arange(256)
    wid[0:4, 128:384] = (np.arange(4)[:, None] == (mcol[None, :] // 64)).astype(np.float32)
    wid[0:4, 384:388] = 1.0
    wid = wid.astype(mnp)
    wb = np.stack(
        [blobs[f"wl{l}"][:, B_OFF : B_OFF + NCH] for l in range(L)], axis=1
    ).reshape(128, L * NCH).astype(np.float32)
    in_maps = []
    for m in range(NCORES):
        shard = np.ascontiguousarray(xTfull[:, m * BC : (m + 1) * BC])
        im = {"wid": wid}
        im.update(wls)
        im["xT"] = shard.astype(mnp) if MODE == "bf16" else shard
        im["xF"] = shard
        if bias_nonzero:
            im["wb"] = wb
        in_maps.append(im)
    from concourse import bass2jax

    results = bass2jax.run_bass_via_pjrt(nc, in_maps, n_cores=NCORES)
    out = np.empty((B, D), np.float32)
    for m in range(NCORES):
        out[m * BC : (m + 1) * BC, :] = results[m]["outT"].T
    return out



# revision 9
# speedup vs baseline: 1.2355x; 1.2355x over previous
"""CrossNetMoE forward on 8 Trainium2 NeuronCores (Bass/Tile).

Math (per layer i, E=4 experts, rank R=64, D=1024):
    v = tanh(V_e @ xl)            [B,E,R]
    c = tanh(C_e @ v_e)           [B,E,R]
    g = softmax(Wg_e . xl)        [B,E]
    u = sum_e (g_e * c_e) @ U_e.T + b      (softmax weights sum to 1)
    xl' = tanh(u * x0 + xl)   (last layer: no tanh)

Strategy: pure data-parallel over batch (2048 rows/core), everything kept in
transposed layout [D, B] on-chip so each layer's matmuls consume the previous
layer's output directly as the PE moving operand; softmax over the 4 experts is
done with tiny auxiliary matmuls (partition reduction / broadcast); the final
`u * x0 + xl` keeps u in a bf16 PSUM bank: DVE multiplies x0 and adds xl in
place (both at 2x 16-bit DVE rate), ACT evacuates with the tanh. The last
layer's result is written bf16 straight from the DVE add and DMA'd per half on
the otherwise-idle Pool DMA queue.

Host side transposes x once and un-transposes the result; weights are packed
host-side into SBUF-image blobs replicated to all cores.
"""
import json
import os
import sys

sys.path.insert(0, "/opt/trn_rl_repo")

import numpy as np

L, E, D, R = 3, 4, 1024, 64
B = 16384
NCORES = 8
BC = B // NCORES          # 2048 rows per core
N = 256                   # batch columns per group (matmul moving free dim)
G = BC // N               # 8 groups per core
NCH = D // 128            # 8 d-chunks

# per-layer weight blob column offsets (f32 columns, [128, COLS_L])
V_OFF = 0                 # 2 groups x 8 chunks x 128
U_OFF = 2048              # 2 kchunks x 8 mchunks x 128
C_OFF = 4096              # 2 groups x 128
W_OFF = 4352              # 8 chunks x 4
B_OFF = 4384              # 8 chunks x 1
COLS_L = 4392
# common blob
I_OFF = 0                 # identity 128
P_OFF = 128               # 2 groups x 128 (partitions 0..3)
O4_OFF = 384              # ones [4,1]
O14_OFF = 385             # ones [1,4]
O44_OFF = 389             # all-ones [4,4] for Z4
COLS_C = 396

_EXEMPT = {"Call"}


def _legalize_json_bytes(raw: bytes) -> bytes:
    """Split multi-wait instructions: walrus allows 1 sync-wait per inst."""
    m = json.loads(raw)
    counter = [0]

    def fix_block(block):
        insts = block.get("instructions")
        if insts is not None:
            out = []
            for inst in insts:
                si = inst.get("sync_info")
                if (
                    si
                    and inst.get("opcode") not in _EXEMPT
                    and len(si.get("on_wait") or []) > 1
                ):
                    for w in si["on_wait"][:-1]:
                        counter[0] += 1
                        out.append(
                            {
                                "name": f"I-waitsplit-{counter[0]}",
                                "opcode": "NoOp",
                                "engine": inst["engine"],
                                "ins": [],
                                "outs": [],
                                "debug": 0,
                                "sync_info": {"on_wait": [w], "on_update": []},
                            }
                        )
                    si["on_wait"] = [si["on_wait"][-1]]
                out.append(inst)
            block["instructions"] = out
        for sub in block.get("blocks") or []:
            fix_block(sub)

    for f in m["functions"]:
        for b in f["blocks"]:
            fix_block(b)
    return json.dumps(m).encode()


def pack_weights(U, V, C, Wg, b):
    """Pack parameters into SBUF-image blobs [128, COLS] (host side)."""
    U, V, C, Wg, b = (np.asarray(a, np.float32) for a in (U, V, C, Wg, b))
    blobs = {}
    p = np.arange(128)
    for l in range(L):
        blob = np.zeros((128, COLS_L), np.float32)
        # Vw: lhsT chunk for vT matmuls: [p=d%128, (g,c,m)]; m -> (e=2g+m//64, r=m%64)
        for g in range(2):
            for c in range(NCH):
                m = np.arange(128)
                # V[l, 2g + m//64, m%64, c*128+p]
                blob[:, V_OFF + (g * 8 + c) * 128 : V_OFF + (g * 8 + c + 1) * 128] = (
                    V[l, 2 * g + m[None, :] // 64, m[None, :] % 64, c * 128 + p[:, None]]
                )
        # Uw: [p=(e,r2)%128 of kchunk, (kc,mc,q)] = U[l, (kc*128+p)//64, mc*128+q, (kc*128+p)%64]
        for kc in range(2):
            for mc in range(NCH):
                q = np.arange(128)
                blob[:, U_OFF + (kc * 8 + mc) * 128 : U_OFF + (kc * 8 + mc + 1) * 128] = (
                    U[l, (kc * 128 + p[:, None]) // 64, mc * 128 + q[None, :], (kc * 128 + p[:, None]) % 64]
                )
        # Cw: blockdiag pairs: [p=(el',r1), (g,j=(el,r2))]
        for g in range(2):
            j = np.arange(128)
            el_p = p[:, None] // 64
            el_j = j[None, :] // 64
            val = C[l, 2 * g + el_j, j[None, :] % 64, p[:, None] % 64]
            blob[:, C_OFF + g * 128 : C_OFF + (g + 1) * 128] = np.where(el_p == el_j, val, 0.0)
        # Ww: [p=d%128, (c,e)]
        for c in range(NCH):
            blob[:, W_OFF + c * 4 : W_OFF + (c + 1) * 4] = Wg[l, :, c * 128 + p]
        # bias
        for c in range(NCH):
            blob[:, B_OFF + c] = b[l, c * 128 + p]
        blobs[f"wl{l}"] = blob
    wc = np.zeros((128, COLS_C), np.float32)
    wc[:, I_OFF : I_OFF + 128] = np.eye(128, dtype=np.float32)
    for g in range(2):
        m = np.arange(128)
        wc[0:4, P_OFF + g * 128 : P_OFF + (g + 1) * 128] = (
            np.arange(4)[:, None] == (2 * g + m[None, :] // 64)
        ).astype(np.float32)
    wc[0:4, O4_OFF] = 1.0
    wc[0:1, O14_OFF : O14_OFF + 4] = 1.0
    wc[0:4, O44_OFF : O44_OFF + 4] = 1.0
    blobs["wc"] = wc
    return blobs


def build_nc(bias_nonzero=False, mode="bf16"):
    import concourse.bass as bass
    import concourse.tile as tile
    from concourse import mybir
    from concourse.tile import add_dep_helper

    f32 = mybir.dt.float32
    AF = mybir.ActivationFunctionType
    ALU = mybir.AluOpType
    bf16 = mybir.dt.bfloat16
    mt = bf16

    nc = bass.Bass()
    xT = nc.dram_tensor("xT", [D, BC], mt, kind="ExternalInput")
    wl = [nc.dram_tensor(f"wl{l}", [128, COLS_L], mt, kind="ExternalInput") for l in range(L)]
    wbd = nc.dram_tensor("wb", [128, L * NCH], f32, kind="ExternalInput") if bias_nonzero else None
    wid = nc.dram_tensor("wid", [128, 388], mt, kind="ExternalInput")
    outT = nc.dram_tensor("outT", [D, BC], mt, kind="ExternalOutput")

    # chain matmuls that share a psum tile so scheduler keeps program order
    last_mm = {}

    def mm(key, out, lhsT, rhs, start, stop):
        inst = nc.tensor.matmul(out, lhsT, rhs, start=start, stop=stop, skip_group_check=True)
        if key in last_mm:
            add_dep_helper(inst.ins, last_mm[key].ins, sync=False, reason="psum order")
        last_mm[key] = inst
        return inst

    with tile.TileContext(nc) as tc:
        with (
            tc.tile_pool(name="wpool", bufs=1) as wpool,
            tc.tile_pool(name="xpool", bufs=1) as xpool,
            tc.tile_pool(name="mid", bufs=1) as mid,
            tc.tile_pool(name="pspool", bufs=1, space="PSUM") as pspool,
            nc.allow_low_precision(reason="bf16 matmul/psum pipeline (intentional)"),
        ):
            wt = []
            for l in range(L):
                w_l = wpool.tile([128, COLS_L], mt, name=f"wt{l}")
                wt.append(w_l)
            wb_f32 = None
            identm = wpool.tile([128, 388], mt, name="identm")
            Pb = identm[0:4, 128:384]
            ones44 = identm[0:4, 384:388]

            xT_v = xT.rearrange("(c p) b -> p c b", p=128)
            outT_q = outT.rearrange("(q c p) b -> p q c b", p=128, q=4)

            ROT = 4  # groups processed in rotation
            for sup in range(G // ROT):
                gs = [ROT * sup + i for i in range(ROT)]
                st = {}
                if sup == 0:
                    # startup: V weights + first x0 group first so PE can start
                    nc.sync.dma_start(wt[0][:, 0:2048], wl[0][:, 0:2048])
                    nc.scalar.dma_start(identm[:], wid[:])
                    for i, g in enumerate(gs):
                        x0t = xpool.tile([128, NCH * N], mt, tag="x0", bufs=8, name=f"x0_{g}")
                        (nc.sync if i == 0 else nc.scalar).dma_start(
                            x0t[:], xT_v[:, :, g * N : (g + 1) * N]
                        )
                        if i == 0:
                            # gating/C/bias columns (small, needed early)
                            nc.sync.dma_start(wt[0][:, 4096:COLS_L], wl[0][:, 4096:COLS_L])
                        st[g] = dict(x0t=x0t, xin=x0t)
                    nc.sync.dma_start(wt[0][:, 2048:4096], wl[0][:, 2048:4096])
                    if bias_nonzero:
                        wbt = wpool.tile([128, L * NCH], f32)
                        nc.sync.dma_start(wbt[:], wbd[:])
                        wb_f32 = [wbt[:, l * NCH : (l + 1) * NCH] for l in range(L)]
                    for l_ in range(1, L):
                        nc.scalar.dma_start(wt[l_][:], wl[l_][:])
                else:
                    for g in gs:
                        x0t = xpool.tile([128, NCH * N], mt, tag="x0", bufs=8, name=f"x0_{g}")
                        nc.sync.dma_start(x0t[:], xT_v[:, :, g * N : (g + 1) * N])
                        st[g] = dict(x0t=x0t, xin=x0t)

                for l in range(L):
                    # ---------- P1 + per-group softmax chain ----------
                    for gi, g in enumerate(gs):
                        S = st[g]
                        xin = S["xin"]
                        pair = gi // 2
                        soff = (gi % 2) * N
                        if gi % 2 == 0:
                            st[("sml", pair)] = pspool.tile(
                                [4, 512], f32, tag="sml", bufs=1, name=f"sml{sup}_{l}_{pair}"
                            )
                        s_ps = st[("sml", pair)]
                        ks = f"s{sup}_{l}_{pair}"
                        kv = f"v{g}_{l}"
                        v_ps = pspool.tile([128, 512], f32, tag="v", bufs=2, name=f"vps{g}_{l}")
                        for c in range(NCH):
                            rhs = xin[:, c * N : (c + 1) * N]
                            mm(kv, v_ps[:, 0:N], wt[l][:, V_OFF + c * 128 : V_OFF + (c + 1) * 128], rhs, start=(c == 0), stop=False)
                            mm(kv, v_ps[:, N : 2 * N], wt[l][:, V_OFF + (8 + c) * 128 : V_OFF + (9 + c) * 128], rhs, start=False, stop=(c == NCH - 1))
                        for c in range(NCH):
                            rhs = xin[:, c * N : (c + 1) * N]
                            mm(ks, s_ps[0:4, soff : soff + N], wt[l][:, W_OFF + c * 4 : W_OFF + (c + 1) * 4], rhs, start=(c == 0 and gi % 2 == 0), stop=(c == NCH - 1))
                        # chain for this group: tanh(v), exp(s), Z4, 1/Z, gates
                        vt = mid.tile([128, 512], mt, tag="vt", bufs=5, name=f"vt{g}_{l}")
                        nc.scalar.activation(vt[:], v_ps[:], AF.Tanh)
                        e_s = mid.tile([4, N], mt, tag="es", bufs=5, name=f"es{g}_{l}")
                        nc.scalar.activation(e_s[:], s_ps[0:4, soff : soff + N], AF.Exp)
                        # Z4: all-ones 4x4 -> each row = sum over experts (overlays s region)
                        mm(ks, s_ps[0:4, soff : soff + N], ones44, e_s[:], start=True, stop=True)
                        rr4 = mid.tile([4, N], mt, tag="rr", bufs=5, name=f"rr{g}_{l}")
                        nc.vector.reciprocal(rr4[:], s_ps[0:4, soff : soff + N])
                        g4 = mid.tile([4, N], mt, tag="g4", bufs=5, name=f"g4{g}_{l}")
                        nc.vector.tensor_mul(g4[:], e_s[:], rr4[:])
                        S["vt"], S["g4"] = vt, g4
                    # ---------- C / gate-broadcast / cg ----------
                    for g in gs:
                        S = st[g]
                        kc = f"c{g}_{l}"
                        c_ps = pspool.tile([128, 512], f32, tag="cb", bufs=1, name=f"cps{g}_{l}")
                        mm(kc, c_ps[:, 0:N], wt[l][:, C_OFF : C_OFF + 128], S["vt"][:, 0:N], start=True, stop=True)
                        mm(kc, c_ps[:, N : 2 * N], wt[l][:, C_OFF + 128 : C_OFF + 256], S["vt"][:, N : 2 * N], start=False, stop=True)
                        ct = mid.tile([128, 512], mt, tag="ct", bufs=4, name=f"ct{g}_{l}")
                        nc.scalar.activation(ct[:], c_ps[:], AF.Tanh)
                        kb = f"b{g}_{l}"
                        b_ps = pspool.tile([128, 512], f32, tag="cb", bufs=1, name=f"bps{g}_{l}")
                        mm(kb, b_ps[:, 0:N], Pb[:, 0:128], S["g4"][:], start=True, stop=True)
                        mm(kb, b_ps[:, N : 2 * N], Pb[:, 128:256], S["g4"][:], start=False, stop=True)
                        cg = mid.tile([128, 512], mt, tag="cg", bufs=4, name=f"cg{g}_{l}")
                        nc.vector.tensor_mul(cg[:], ct[:], b_ps[:])
                        S["cg"] = cg
                        if l < L - 1:
                            S["xout"] = xpool.tile([128, NCH * N], mt, tag="xl", bufs=8, name=f"xl{g}_{l}")
                        else:
                            S["xout"] = xpool.tile([128, NCH * N], mt, tag="osb", bufs=3, name=f"osb{g}")
                    # ---------- U matmuls + epilogue (quarter granularity) ----------
                    for g in gs:
                        S = st[g]
                        xin, x0t, cg, xout = S["xin"], S["x0t"], S["cg"], S["xout"]
                        for q in range(4):
                            ku = f"u{g}_{l}_{q}"
                            u_ps = pspool.tile([128, 512], f32, tag="u", bufs=4, name=f"ups{g}_{l}_{q}")
                            for mi, mc in enumerate((2 * q, 2 * q + 1)):
                                col = mi * N
                                for kch in range(2):
                                    mm(
                                        ku,
                                        u_ps[:, col : col + N],
                                        wt[l][:, U_OFF + (kch * 8 + mc) * 128 : U_OFF + (kch * 8 + mc + 1) * 128],
                                        cg[:, kch * N : (kch + 1) * N],
                                        start=(mi == 0 and kch == 0),
                                        stop=(kch == 1),
                                    )
                            qs = slice(q * 2 * N, (q + 1) * 2 * N)
                            if bias_nonzero:
                                for mi, mc in enumerate((2 * q, 2 * q + 1)):
                                    col = mi * N
                                    nc.vector.scalar_tensor_tensor(
                                        xout[:, mc * N : (mc + 1) * N],
                                        u_ps[:, col : col + N],
                                        wb_f32[l][:, mc : mc + 1],
                                        x0t[:, mc * N : (mc + 1) * N],
                                        ALU.add,
                                        ALU.mult,
                                    )
                            else:
                                # evacuate PSUM: xout_q = u_ps * x0 (bf16 SBUF out)
                                nc.vector.tensor_mul(xout[:, qs], u_ps[:], x0t[:, qs])
                            # + xl alternating DVE (2x bf16) / Pool to balance engines
                            eng = nc.vector if q % 2 == 0 else nc.gpsimd
                            eng.tensor_add(xout[:, qs], xout[:, qs], xin[:, qs])
                            if l < L - 1:
                                nc.scalar.activation(xout[:, qs], xout[:, qs], AF.Tanh)
                            else:
                                nc.sync.dma_start(
                                    outT_q[:, q, :, g * N : (g + 1) * N],
                                    xout[:, qs].rearrange("p (c n) -> p c n", c=2),
                                )
                    for g in gs:
                        st[g]["xin"] = st[g]["xout"]

    # walrus wait-budget legalization on serialization
    orig = nc.to_json_bytes
    nc.to_json_bytes = lambda: _legalize_json_bytes(orig())
    return nc


_CACHE = {}


MODE = "bf16"


def kernel(x, U, V, C, Wg, b):
    import ml_dtypes

    x = np.ascontiguousarray(np.asarray(x, np.float32))
    bias_nonzero = bool(np.any(np.asarray(b) != 0))
    key = ("nc", bias_nonzero, MODE)
    if key not in _CACHE:
        _CACHE[key] = build_nc(bias_nonzero, MODE)
    nc = _CACHE[key]
    mnp = ml_dtypes.bfloat16
    blobs = pack_weights(U, V, C, Wg, b)
    xTfull = np.ascontiguousarray(x.T)  # [D, B]
    wls = {f"wl{l}": np.ascontiguousarray(blobs[f"wl{l}"].astype(mnp)) for l in range(L)}
    wid = np.zeros((128, 388), np.float32)
    wid[:, 0:128] = np.eye(128, dtype=np.float32)
    mcol = np.arange(256)
    wid[0:4, 128:384] = (np.arange(4)[:, None] == (mcol[None, :] // 64)).astype(np.float32)
    wid[0:4, 384:388] = 1.0
    wid = wid.astype(mnp)
    wb = np.stack(
        [blobs[f"wl{l}"][:, B_OFF : B_OFF + NCH] for l in range(L)], axis=1
    ).reshape(128, L * NCH).astype(np.float32)
    in_maps = []
    for m in range(NCORES):
        shard = np.ascontiguousarray(xTfull[:, m * BC : (m + 1) * BC])
        im = {"wid": wid}
        im.update(wls)
        im["xT"] = shard.astype(mnp)
        if bias_nonzero:
            im["wb"] = wb
        in_maps.append(im)
    from concourse import bass2jax

    results = bass2jax.run_bass_via_pjrt(nc, in_maps, n_cores=NCORES)
    out = np.empty((B, D), np.float32)
    for m in range(NCORES):
        out[m * BC : (m + 1) * BC, :] = results[m]["outT"].T.astype(np.float32)
    return out
